# revision 1
# baseline (speedup 1.0000x reference)
"""Trainium2 Bass kernel for nn_NewActivationGNN (GNN message passing).

Self-contained: hardcodes problem shapes. Distributes across 8 NeuronCores:
nodes sharded by destination (graph parallel), per-layer AllGather of the
hidden-state table, per-edge gathers via the custom dma_gather instruction,
and segment-sum via per-chunk selection-matrix matmuls accumulated in PSUM
(feature-major packs, one PSUM bank per 512 destination columns).

SPMD: one program runs on all 8 cores; all per-device variation (indices,
selection matrices, features) is data. The compile-time chunk schedule is
made device-uniform by padding per-window slot budgets to the cross-device
maximum.
"""

import sys

for _p in ("/opt/trn_rl_repo", "/root/.axon_site/_ro/trn_rl_repo"):
    if _p not in sys.path:
        sys.path.insert(0, _p)

import math
from dataclasses import dataclass

import numpy as np

import concourse.bass as bass  # noqa: F401
import concourse.tile as tile
from concourse import bacc, mybir
from concourse.masks import make_identity

P = 128


@dataclass
class Cfg:
    N: int = 50000
    E: int = 800000
    NFEAT: int = 500
    NHID: int = 128
    NCLASS: int = 40
    NLAYERS: int = 4
    GAMMA: float = 0.3
    X1: float = 0.1
    X2: float = 0.9
    C_ACT: float = -1.0
    n_cores: int = 8
    WIN: int = 32            # dest columns per window
    dt: str = "float16"      # gather-table / matmul operand dtype

    @property
    def R(self):
        return self.N // self.n_cores

    @property
    def T_SPLIT(self):       # lo/hi split of the permuted table (5 devices)
        return 5 * self.R

    @property
    def NPACK(self):
        return math.ceil(self.R / 512)

    @property
    def NQ(self):
        return math.ceil(self.R / P)

    @property
    def NFP(self):
        return ((self.NFEAT + 1 + P - 1) // P) * P

    @property
    def np_dt(self):
        return np.float16 if self.dt == "float16" else np.float32

    @property
    def my_dt(self):
        return mybir.dt.float16 if self.dt == "float16" else mybir.dt.float32


class Sched:
    """Device-uniform compile-time schedule derived from cross-device-max
    window slot budgets W[pack, win, pass]."""

    def __init__(self, cfg: Cfg, W: np.ndarray):
        self.W = W                       # [NPACK, 16, 2] multiples not needed
        self.chunks = []                 # [pk][ps] -> list of (col_off, width, g_col)
        self.s_off = []                  # [pk] -> per-chunk S column offset (global)
        self.s_width = []                # [pk] -> per-chunk S width
        self.s_total = 0
        self.g_cols = []                 # [pk] total gather columns
        self.g_lo_cols = []              # [pk] lo gather columns
        self.idx_off = []                # [2*pk+ps] slot-stream offset
        self.idx_total = 0
        idx_cum = 0
        for pk in range(cfg.NPACK):
            self.chunks.append([[], []])
            self.s_off.append([])
            self.s_width.append([])
            g_col = 0
            for ps in range(2):
                n_slots = int(W[pk, :, ps].sum())
                n_pad = ((n_slots + P - 1) // P) * P
                bounds = np.cumsum(np.concatenate([[0], W[pk, :, ps]]))
                bounds[-1] = n_pad
                for c in range(n_pad // P):
                    s0, s1 = c * P, (c + 1) * P
                    w0 = min(int(np.searchsorted(bounds, s0, side="right")) - 1, 15)
                    w1 = min(int(np.searchsorted(bounds, s1 - 1, side="right")) - 1, 15)
                    col_off = w0 * cfg.WIN
                    width = min((w1 - w0 + 1) * cfg.WIN, 512 - col_off)
                    self.chunks[pk][ps].append((col_off, width, g_col))
                    self.s_off[pk].append(self.s_total)
                    self.s_width[pk].append(width)
                    self.s_total += width
                    g_col += 1
                self.idx_off.append(idx_cum)
                idx_cum += n_pad
                if ps == 0:
                    self.g_lo_cols.append(n_pad // P)
            self.g_cols.append(g_col)
        self.idx_total = idx_cum


def preprocess(cfg: Cfg, features, edge_row, edge_col, W_in, Ws, c, W_out):
    """Host-side preprocessing -> (in_maps, sched, perm)."""
    import heapq

    N, R, nc_ = cfg.N, cfg.R, cfg.n_cores
    f32 = np.float32
    deg = np.bincount(edge_row, minlength=N)
    deg_inv = (1.0 / np.maximum(deg, 1)).astype(f32)

    owner = edge_row // R
    is_hi = (edge_col // R) >= 5         # source in hi half of table

    n_win = 16 * cfg.NPACK
    n_full_win = 16 * (cfg.NPACK - 1)
    tail = R - 512 * (cfg.NPACK - 1)

    dest_of = np.empty((nc_, R), np.int64)    # local id -> orig local dest
    local_of = np.empty((nc_, R), np.int64)   # orig local dest -> local id
    load = np.zeros((nc_, cfg.NPACK, 16, 2), np.int64)
    dev_edges = []

    for d in range(nc_):
        m = owner == d
        erow = edge_row[m] - d * R
        dev_edges.append((erow, edge_col[m], is_hi[m]))
        dlo = np.bincount(erow[~is_hi[m]], minlength=R)
        dhi = np.bincount(erow[is_hi[m]], minlength=R)
        tot = dlo + dhi
        order = np.argsort(-tot, kind="stable")
        tail_d = order[-tail:] if tail > 0 else np.array([], np.int64)
        main_d = order[:len(order) - tail]
        wd = [[] for _ in range(n_win)]
        if n_full_win > 0:
            heap = [(0, w) for w in range(n_full_win)]
            heapq.heapify(heap)
            counts = [0] * n_full_win
            for o in main_d:
                while True:
                    ld, w = heapq.heappop(heap)
                    if counts[w] < cfg.WIN:
                        break
                wd[w].append(int(o))
                counts[w] += 1
                if counts[w] < cfg.WIN:
                    heapq.heappush(heap, (ld + int(tot[o]), w))
        for i, o in enumerate(tail_d):
            wd[n_full_win + i // cfg.WIN].append(int(o))
        # rank-sort full-pack windows by load desc within each pack
        for pk in range(cfg.NPACK - 1):
            ws = wd[16 * pk:16 * (pk + 1)]
            key = [-sum(int(tot[o]) for o in w) for w in ws]
            wd[16 * pk:16 * (pk + 1)] = [ws[i] for i in np.argsort(key, kind="stable")]
        for w in range(n_win):
            pk, wi = divmod(w, 16)
            for j, o in enumerate(wd[w]):
                li = 512 * pk + cfg.WIN * wi + j
                local_of[d, o] = li
                dest_of[d, li] = o
            load[d, pk, wi, 0] = int(dlo[wd[w]].sum()) if wd[w] else 0
            load[d, pk, wi, 1] = int(dhi[wd[w]].sum()) if wd[w] else 0

    sch = Sched(cfg, load.max(axis=0))
    W = sch.W

    pos = np.empty(N, np.int64)
    for d in range(nc_):
        pos[d * R + np.arange(R)] = d * R + local_of[d]

    # window slot-stream bases (uniform across devices)
    win_base = np.zeros((cfg.NPACK, 16, 2), np.int64)
    for pk in range(cfg.NPACK):
        for ps in range(2):
            win_base[pk, :, ps] = (sch.idx_off[2 * pk + ps] +
                                   np.concatenate([[0], np.cumsum(W[pk, :, ps])[:-1]]))
    # per-(pass-chunk) metadata as arrays for vectorized S fill
    ch_base = []   # [pk][ps] global chunk index of first chunk
    g = 0
    for pk in range(cfg.NPACK):
        ch_base.append([g, g + len(sch.chunks[pk][0])])
        g += len(sch.chunks[pk][0]) + len(sch.chunks[pk][1])
    all_co = np.array([co for pk in range(cfg.NPACK) for ps in range(2)
                       for (co, cw, gc) in sch.chunks[pk][ps]], np.int64)
    all_soff = np.array([o for pk in range(cfg.NPACK) for o in sch.s_off[pk]],
                        np.int64)

    # shared weights
    NFP = cfg.NFP
    W_aug = np.zeros((NFP, cfg.NHID), f32)
    W_aug[:cfg.NFEAT] = (1.0 - cfg.GAMMA) * W_in
    W_aug[cfg.NFEAT] = cfg.GAMMA * np.maximum(c, 0.0)
    nk = NFP // P
    W_dram = np.empty((P, nk * P), cfg.np_dt)
    for k in range(nk):
        W_dram[:, k * P:(k + 1) * P] = W_aug[k * P:(k + 1) * P]
    Ws_dram = np.empty((P, cfg.NLAYERS * P), cfg.np_dt)
    for l in range(cfg.NLAYERS):
        Ws_dram[:, l * P:(l + 1) * P] = Ws[l]
    Wout_dram = np.ascontiguousarray(W_out).astype(cfg.np_dt)

    in_maps = []
    for d in range(nc_):
        erow, ecol, ehi = dev_edges[d]
        li = local_of[d][erow]
        ps = ehi.astype(np.int64)
        key = li * 2 + ps
        order = np.argsort(key, kind="stable")
        key_s = key[order]
        ecol_s = ecol[order]
        cnt = np.bincount(key_s, minlength=2 * R)
        goff = np.concatenate([[0], np.cumsum(cnt)])
        idx_in_grp = np.arange(len(key_s)) - goff[key_s]
        # dest offset within its window's stream (per pass)
        cnt2 = cnt.reshape(R, 2)
        cnt_pad = np.zeros((cfg.NPACK * 512, 2), np.int64)
        cnt_pad[:R] = cnt2
        cw = cnt_pad.reshape(cfg.NPACK * 16, cfg.WIN, 2)
        dest_off = (np.cumsum(cw, axis=1) - cw).reshape(cfg.NPACK * 512, 2)
        li_s = key_s // 2
        ps_s = key_s % 2
        pk_s = li_s // 512
        wi_s = (li_s % 512) // cfg.WIN
        spos = (win_base[pk_s, wi_s, ps_s] + dest_off[li_s, ps_s] + idx_in_grp)
        # gather index values
        pv = pos[ecol_s]
        pv = np.where(ps_s == 1, pv - cfg.T_SPLIT, pv)
        idx_vals = np.zeros(sch.idx_total, np.int16)
        idx_vals[spos] = pv.astype(np.int16)
        # S fill: chunk of each slot & column within chunk
        seg_off = np.array(sch.idx_off + [sch.idx_total], np.int64)
        seg_id = 2 * pk_s + ps_s
        s_rel = spos - seg_off[seg_id]
        cch = np.array([ch_base[pk][ps] for pk in range(cfg.NPACK)
                        for ps in range(2)], np.int64)[seg_id] + s_rel // P
        srow = s_rel % P
        col_in_pack = li_s % 512
        scol = all_soff[cch] + (col_in_pack - all_co[cch])
        s_data = np.zeros((P, sch.s_total), cfg.np_dt)
        s_data[srow, scol] = deg_inv[d * R + dest_of[d][li_s]]
        # wrap idx into [16, total/16] segments then replicate to 128
        idx_t = np.zeros((16, sch.idx_total // 16), np.int16)
        for gi in range(2 * cfg.NPACK):
            b, e = seg_off[gi], seg_off[gi + 1]
            if e > b:
                idx_t[:, b // 16:e // 16] = idx_vals[b:e].reshape(-1, 16).T
        idx_t = np.tile(idx_t, (8, 1))

        gids = d * R + dest_of[d]
        featT = np.zeros((NFP, R), cfg.np_dt)
        featT[:cfg.NFEAT] = features[gids].T
        featT[cfg.NFEAT] = 1.0

        in_maps.append(dict(
            featT=featT, idx_all=np.ascontiguousarray(idx_t),
            s_all=s_data, w_proj=W_dram, w_hid=Ws_dram, w_out=Wout_dram,
        ))

    perm = np.concatenate([d * R + dest_of[d] for d in range(nc_)])
    return in_maps, sch, perm


def build_program(cfg: Cfg, sch: Sched, enable_asserts=False, rep=1, no_coll=False):
    import os
    skip = set(os.environ.get("GNN_SKIP", "").split(","))
    nc = bacc.Bacc("TRN2", target_bir_lowering=False, debug=False,
                   enable_asserts=enable_asserts,
                   num_devices=1 if no_coll else cfg.n_cores,
                   num_swdge_queues=4)
    DT = cfg.my_dt
    f32 = mybir.dt.float32
    R, NQ, NPACK, NFP = cfg.R, cfg.NQ, cfg.NPACK, cfg.NFP
    AFT = mybir.ActivationFunctionType
    ALU = mybir.AluOpType
    AX = mybir.AxisListType
    rg = [list(range(cfg.n_cores))]
    nk = NFP // P
    nc._gq = 0

    featT = nc.dram_tensor("featT", [NFP, R], DT, kind="ExternalInput").ap()
    idx_all = nc.dram_tensor("idx_all", [P, sch.idx_total // 16],
                             mybir.dt.int16, kind="ExternalInput").ap()
    s_all = nc.dram_tensor("s_all", [P, sch.s_total], DT,
                           kind="ExternalInput").ap()
    w_proj = nc.dram_tensor("w_proj", [P, nk * P], DT,
                            kind="ExternalInput").ap()
    w_hid = nc.dram_tensor("w_hid", [P, cfg.NLAYERS * P], DT,
                           kind="ExternalInput").ap()
    w_out = nc.dram_tensor("w_out", [P, cfg.NCLASS], DT,
                           kind="ExternalInput").ap()
    out = nc.dram_tensor("out", [R, cfg.NCLASS], f32,
                         kind="ExternalOutput").ap()

    INV08 = float(np.float32(1.0 / (np.float64(cfg.X2) - cfg.X1 + 1e-8)))
    B_RELU = float(np.float32(-cfg.X1 * INV08))
    E1 = float(1.0 + np.exp(-cfg.C_ACT))

    with tile.TileContext(nc) as tc:
        with tc.tile_pool(name="persist", bufs=1) as persist, \
             tc.tile_pool(name="dram", bufs=1, space="DRAM") as dram:
            # ---- persistent tiles ----
            idx_sb = persist.tile([P, sch.idx_total // 16], mybir.dt.int16)
            nc.sync.dma_start(idx_sb[:], idx_all[:])
            x0_sb = persist.tile([P, NQ * P], f32)
            wh_sb = persist.tile([P, cfg.NLAYERS * P], DT)
            nc.sync.dma_start(wh_sb[:], w_hid[:])
            wo_sb = persist.tile([P, cfg.NCLASS], DT)
            nc.sync.dma_start(wo_sb[:], w_out[:])
            wp_sb = persist.tile([P, nk * P], DT)
            nc.sync.dma_start(wp_sb[:], w_proj[:])
            zero1 = persist.tile([1, P], DT)
            nc.vector.memset(zero1[:], 0.0)
            zero512 = persist.tile([1, 512], DT)
            nc.vector.memset(zero512[:], 0.0)
            ones1 = persist.tile([1, P], f32)
            nc.vector.memset(ones1[:], 1.0)
            b_relu = persist.tile([P, 1], f32)
            nc.vector.memset(b_relu[:], B_RELU)
            idn = persist.tile([P, P], f32)
            make_identity(nc, idn[:])
            rmax = persist.tile([P, 1], f32)
            rmin = persist.tile([P, 1], f32)
            mm_sb = persist.tile([P, 2], f32)
            mm_red = persist.tile([1, 2], f32)
            mm_back = persist.tile([1, 2], f32)
            sfac = persist.tile([P, 1], f32)
            bfac = persist.tile([P, 1], f32)

            NIT = cfg.NLAYERS * rep
            x_full = [dram.tile([cfg.N, cfg.NHID], DT, addr_space="Shared",
                                name=f"x_full{i}") for i in range(NIT)]
            bounce = [dram.tile([R, cfg.NHID], DT, name=f"bounce{i}")
                      for i in range(NIT)]
            mm_in = dram.tile([1, 2], f32)
            mm_out = dram.tile([1, 2], f32, addr_space="Shared")

            # ================= projection phase =================
            with tc.tile_pool(name="strips", bufs=1) as strip_pool, \
                 tc.tile_pool(name="pwork", bufs=2) as pwork, \
                 tc.tile_pool(name="pps", bufs=2, space="PSUM") as pps_pool:
                strips = []
                for k in range(nk):
                    st = strip_pool.tile([P, R], DT, name=f"strip{k}",
                                         tag=f"strip{k}")
                    nc.sync.dma_start(st[:], featT[k * P:(k + 1) * P, :])
                    strips.append(st)
                for q in range(NQ):
                    r0 = q * P
                    w = min(P, R - r0)
                    h0ps = pps_pool.tile([P, P], f32, name="h0ps", tag="h0ps")
                    for k in range(nk):
                        nc.tensor.matmul(h0ps[:w, :], lhsT=strips[k][:, r0:r0 + w],
                                         rhs=wp_sb[:, k * P:(k + 1) * P],
                                         start=(k == 0), stop=(k == nk - 1))
                    nc.vector.tensor_copy(x0_sb[:w, q * P:(q + 1) * P],
                                          h0ps[:w, :])
                    qmax = pwork.tile([P, 1], f32, name="qmax", tag="qmax")
                    qmin = pwork.tile([P, 1], f32, name="qmin", tag="qmin")
                    nc.vector.tensor_reduce(qmax[:w], h0ps[:w, :], axis=AX.X,
                                            op=ALU.max)
                    nc.vector.tensor_reduce(qmin[:w], h0ps[:w, :], axis=AX.X,
                                            op=ALU.min)
                    if q == 0:
                        nc.vector.tensor_copy(rmax[:], qmax[:])
                        nc.vector.tensor_copy(rmin[:], qmin[:])
                    else:
                        nc.vector.tensor_tensor(rmax[:w], rmax[:w], qmax[:w],
                                                op=ALU.max)
                        nc.vector.tensor_tensor(rmin[:w], rmin[:w], qmin[:w],
                                                op=ALU.min)
                nc.vector.tensor_copy(mm_sb[:, 0:1], rmax[:])
                nc.vector.tensor_scalar(mm_sb[:, 1:2], rmin[:], -1.0, None,
                                        ALU.mult)
                nc.gpsimd.tensor_reduce(mm_red[:], mm_sb[:], axis=AX.C,
                                        op=ALU.max)
                nc.sync.dma_start(mm_in[:], mm_red[:])
                if no_coll:
                    nc.sync.dma_start(mm_back[:], mm_in[:])
                else:
                    nc.gpsimd.collective_compute(
                        "AllReduce", ALU.max, ins=[mm_in.opt()],
                        outs=[mm_out.opt()], replica_groups=rg)
                    nc.sync.dma_start(mm_back[:], mm_out[:])
                bc_ps = pps_pool.tile([P, 2], f32, name="bc_ps", tag="h0ps")
                nc.tensor.matmul(bc_ps[:], lhsT=ones1[:], rhs=mm_back[:],
                                 start=True, stop=True)
                bcast = pwork.tile([P, 2], f32, name="bcast", tag="qmin")
                nc.vector.tensor_copy(bcast[:], bc_ps[:])
                sden = pwork.tile([P, 1], f32, name="sden", tag="qmax")
                nc.vector.tensor_tensor(sden[:], bcast[:, 0:1], bcast[:, 1:2],
                                        op=ALU.add)
                nc.vector.tensor_scalar(sden[:], sden[:], 1e-8, None, ALU.add)
                nc.vector.reciprocal(sfac[:], sden[:])
                nc.vector.tensor_tensor(bfac[:], bcast[:, 1:2], sfac[:],
                                        op=ALU.mult)
                for q in range(NQ):
                    r0 = q * P
                    w = min(P, R - r0)
                    sl = slice(q * P, (q + 1) * P)
                    nc.vector.tensor_scalar(x0_sb[:w, sl], x0_sb[:w, sl],
                                            sfac[:w, :], bfac[:w, :],
                                            ALU.mult, ALU.add)
                    xq = pwork.tile([P, P], DT, name="xq", tag="xq")
                    nc.scalar.activation(xq[:w, :], x0_sb[:w, sl], AFT.Copy)
                    nc.sync.dma_start(bounce[0][r0:r0 + w, :], xq[:w, :])
            if no_coll:
                nc.sync.dma_start(x_full[0][:R, :], bounce[0][:])
            else:
                nc.gpsimd.collective_compute(
                    "AllGather", ALU.bypass, ins=[bounce[0].opt()],
                    outs=[x_full[0].opt()], replica_groups=rg)

            # ================= conv layers =================
            with tc.tile_pool(name="gpool", bufs=2) as gpool, \
                 tc.tile_pool(name="spool", bufs=2) as spool, \
                 tc.tile_pool(name="lwork", bufs=3) as work, \
                 tc.tile_pool(name="xnp", bufs=1) as xnp, \
                 tc.tile_pool(name="pack_ps", bufs=2, space="PSUM") as pack_ps, \
                 tc.tile_pool(name="z2_ps", bufs=2, space="PSUM") as z2_ps, \
                 tc.tile_pool(name="lg_ps", bufs=2, space="PSUM") as lg_ps:
                for l in range(cfg.NLAYERS * rep):
                    li = l % cfg.NLAYERS
                    last = l == cfg.NLAYERS * rep - 1
                    beta = min(0.5, (li + 1) / cfg.NLAYERS * 0.5)
                    c1 = float((1.0 - beta) * E1)
                    tbl = x_full[l]
                    x0b = xnp.tile([P, NQ * P], f32, name=f"x0b{l}", tag="x0b")
                    for q in range(NQ):
                        w = min(P, R - q * P)
                        sl = slice(q * P, (q + 1) * P)
                        nc.vector.tensor_scalar(x0b[:w, sl], x0_sb[:w, sl],
                                                float(beta), None, ALU.mult)
                    if not last:
                        xn = xnp.tile([P, NQ * P], DT, name=f"xn{l}", tag="xn")
                    for pk in range(NPACK):
                        ncol = sch.g_cols[pk]
                        nlo = sch.g_lo_cols[pk]
                        gt = gpool.tile([P, max(ncol, 1), P], DT,
                                        name=f"g{l}_{pk}", tag="g")
                        i0 = sch.idx_off[2 * pk] // 16
                        n_lo = nlo * P
                        n_hi = (ncol - nlo) * P
                        if n_lo and "gather" not in skip:
                            nc.gpsimd.dma_gather(
                                out_ap=gt[:, :nlo, :],
                                in_ap=tbl[:cfg.T_SPLIT, :],
                                idxs_ap=idx_sb[:, i0:i0 + n_lo // 16],
                                num_idxs=n_lo, num_idxs_reg=n_lo,
                                elem_size=cfg.NHID, single_packet=False)
                        if n_hi:
                            i1 = sch.idx_off[2 * pk + 1] // 16
                            nc.gpsimd.dma_gather(
                                out_ap=gt[:, nlo:, :],
                                in_ap=tbl[cfg.T_SPLIT:, :],
                                idxs_ap=idx_sb[:, i1:i1 + n_hi // 16],
                                num_idxs=n_hi, num_idxs_reg=n_hi,
                                elem_size=cfg.NHID, single_packet=False)
                        so = sch.s_off[pk][0] if sch.s_off[pk] else 0
                        s_w = sum(sch.s_width[pk])
                        if s_w and "sload" not in skip:
                            s_sb = spool.tile([P, s_w], DT, name=f"s{l}_{pk}",
                                              tag="s")
                            nc.sync.dma_start(s_sb[:], s_all[:, so:so + s_w])
                        elif s_w:
                            s_sb = spool.tile([P, s_w], DT, name=f"s{l}_{pk}",
                                              tag="s")
                        pps = pack_ps.tile([P, 512], f32, name=f"pps{l}_{pk}",
                                           tag="pps")
                        n_ch = (len(sch.chunks[pk][0]) + len(sch.chunks[pk][1])
                                if "chunks" not in skip else 0)
                        nc.tensor.matmul(pps[:], lhsT=zero1[:], rhs=zero512[:],
                                         start=True, stop=(n_ch == 0),
                                         skip_group_check=True)
                        ci = 0
                        chunk_sched = sch.chunks if "chunks" not in skip else [[[], []]] * cfg.NPACK
                        for ps in range(2):
                            for (co, cw, gc) in chunk_sched[pk][ps]:
                                s0 = sch.s_off[pk][ci] - so
                                nc.tensor.matmul(
                                    pps[:, co:co + cw], lhsT=gt[:, gc, :],
                                    rhs=s_sb[:, s0:s0 + cw],
                                    start=False, stop=(ci == n_ch - 1),
                                    skip_group_check=True)
                                ci += 1
                        sT = work.tile([P, 512], DT, name="sT", tag="sT")
                        nc.vector.tensor_copy(sT[:], pps[:])
                        for qq in range(4):
                            q = 4 * pk + qq
                            r0 = q * P
                            if r0 >= R:
                                break
                            w = min(P, R - r0)
                            sl = slice(q * P, (q + 1) * P)
                            z2 = z2_ps.tile([P, P], f32, name="z2", tag="z2")
                            nc.tensor.matmul(z2[:w, :],
                                             lhsT=sT[:, qq * P:qq * P + w],
                                             rhs=wh_sb[:, li * P:(li + 1) * P],
                                             start=True, stop=True)
                            a1 = work.tile([P, P], f32, name="a1", tag="a1")
                            nc.scalar.activation(a1[:w], z2[:w, :], AFT.Relu,
                                                 bias=b_relu[:w], scale=INV08)
                            nc.vector.tensor_scalar(a1[:w], a1[:w], 1.0, c1,
                                                    ALU.min, ALU.mult)
                            a3 = work.tile([P, P], f32, name="a3", tag="a3")
                            nc.scalar.activation(a3[:w], a1[:w], AFT.Sigmoid,
                                                 scale=float(-1.0 / c1))
                            nc.vector.tensor_tensor(a1[:w], a1[:w], a3[:w],
                                                    op=ALU.mult)
                            if not last:
                                nc.vector.tensor_tensor(xn[:w, sl], a1[:w],
                                                        x0b[:w, sl], op=ALU.add)
                                nc.sync.dma_start(bounce[l + 1][r0:r0 + w, :],
                                                  xn[:w, sl])
                            else:
                                x4 = work.tile([P, P], f32, name="x4", tag="a3")
                                nc.vector.tensor_tensor(x4[:w], a1[:w],
                                                        x0b[:w, sl], op=ALU.add)
                                tps = z2_ps.tile([P, P], f32, name="tps",
                                                 tag="z2")
                                nc.tensor.transpose(tps[:, :w], x4[:w],
                                                    identity=idn[:w, :w])
                                x4T = work.tile([P, P], DT, name="x4T",
                                                tag="a1")
                                nc.vector.tensor_copy(x4T[:, :w], tps[:, :w])
                                lg = lg_ps.tile([P, cfg.NCLASS], f32,
                                                name="lg", tag="lg")
                                nc.tensor.matmul(lg[:w, :], lhsT=x4T[:, :w],
                                                 rhs=wo_sb[:], start=True,
                                                 stop=True)
                                mx = work.tile([P, 1], f32, name="mx", tag="mx")
                                nc.vector.tensor_reduce(mx[:w], lg[:w, :],
                                                        axis=AX.X, op=ALU.max)
                                nmx = work.tile([P, 1], f32, name="nmx",
                                                tag="nmx")
                                nc.vector.tensor_scalar(nmx[:w], mx[:w], -1.0,
                                                        None, ALU.mult)
                                ex = work.tile([P, cfg.NCLASS], f32, name="ex",
                                               tag="ex")
                                sume = work.tile([P, 1], f32, name="sume",
                                                 tag="sume")
                                nc.scalar.activation(ex[:w], lg[:w, :], AFT.Exp,
                                                     bias=nmx[:w],
                                                     accum_out=sume[:w])
                                lse = work.tile([P, 1], f32, name="lse",
                                                tag="lse")
                                nc.scalar.activation(lse[:w], sume[:w], AFT.Ln)
                                nc.vector.tensor_tensor(lse[:w], lse[:w],
                                                        nmx[:w], op=ALU.subtract)
                                res = work.tile([P, cfg.NCLASS], f32,
                                                name="res", tag="ex")
                                nc.vector.tensor_scalar(res[:w], lg[:w, :],
                                                        lse[:w], None,
                                                        ALU.subtract)
                                nc.sync.dma_start(out[r0:r0 + w, :], res[:w])
                    if not last:
                        if no_coll:
                            nc.sync.dma_start(x_full[l + 1][:R, :],
                                              bounce[l + 1][:])
                        else:
                            nc.gpsimd.collective_compute(
                                "AllGather", ALU.bypass,
                                ins=[bounce[l + 1].opt()],
                                outs=[x_full[l + 1].opt()],
                                replica_groups=rg)
    nc.compile()
    return nc


def kernel(**inputs) -> np.ndarray:
    cfg = Cfg()
    features = np.asarray(inputs["features"], np.float32)
    edge_row = np.asarray(inputs["edge_row"], np.int64)
    edge_col = np.asarray(inputs["edge_col"], np.int64)
    W_in = np.asarray(inputs["W_in"], np.float32)
    Ws = np.asarray(inputs["Ws"], np.float32)
    c = np.asarray(inputs["c"], np.float32)
    W_out = np.asarray(inputs["W_out"], np.float32)

    in_maps, sch, perm = preprocess(cfg, features, edge_row, edge_col,
                                    W_in, Ws, c, W_out)
    nc = build_program(cfg, sch)

    import os
    from concourse import bass_utils
    res = bass_utils.run_bass_kernel_spmd(
        nc, in_maps, core_ids=list(range(cfg.n_cores)),
        trace=bool(os.environ.get("GNN_TRACE")))
    kernel.last_result = res
    out = np.empty((cfg.N, cfg.NCLASS), np.float32)
    for d in range(cfg.n_cores):
        out[perm[d * cfg.R:(d + 1) * cfg.R]] = res.results[d]["out"]
    return out



# revision 2
# speedup vs baseline: 1.7592x; 1.7592x over previous
"""Trainium2 Bass kernel for nn_NewActivationGNN (GNN message passing).

Self-contained: hardcodes problem shapes. Distributes across 8 NeuronCores:
nodes sharded by destination (graph parallel), per-layer AllGather of the
hidden-state table, per-edge gathers via the custom dma_gather instruction,
and segment-sum via per-chunk selection-matrix matmuls accumulated in PSUM
(feature-major packs, one PSUM bank per 512 destination columns).

SPMD: one program runs on all 8 cores; all per-device variation (indices,
selection matrices, features) is data. The compile-time chunk schedule is
made device-uniform by padding per-window slot budgets to the cross-device
maximum.
"""

import sys

for _p in ("/opt/trn_rl_repo", "/root/.axon_site/_ro/trn_rl_repo"):
    if _p not in sys.path:
        sys.path.insert(0, _p)

import math
from dataclasses import dataclass

import numpy as np

import concourse.bass as bass  # noqa: F401
import concourse.tile as tile
from concourse import bacc, mybir
from concourse.masks import make_identity

P = 128


@dataclass
class Cfg:
    N: int = 50000
    E: int = 800000
    NFEAT: int = 500
    NHID: int = 128
    NCLASS: int = 40
    NLAYERS: int = 4
    GAMMA: float = 0.3
    X1: float = 0.1
    X2: float = 0.9
    C_ACT: float = -1.0
    n_cores: int = 8
    WIN: int = 32            # dest columns per window
    dt: str = "float16"      # gather-table / matmul operand dtype

    @property
    def R(self):
        return self.N // self.n_cores

    @property
    def T_SPLIT(self):       # lo/hi split of the permuted table (5 devices)
        return 5 * self.R

    @property
    def NPACK(self):
        return math.ceil(self.R / 512)

    @property
    def NQ(self):
        return math.ceil(self.R / P)

    @property
    def NFP(self):
        return ((self.NFEAT + 1 + P - 1) // P) * P

    @property
    def np_dt(self):
        return np.float16 if self.dt == "float16" else np.float32

    @property
    def my_dt(self):
        return mybir.dt.float16 if self.dt == "float16" else mybir.dt.float32


class Sched:
    """Device-uniform compile-time schedule derived from cross-device-max
    window slot budgets W[pack, win, pass]."""

    def __init__(self, cfg: Cfg, W: np.ndarray):
        self.W = W                       # [NPACK, 16, 2] multiples not needed
        self.chunks = []                 # [pk][ps] -> list of (col_off, width, g_col)
        self.s_off = []                  # [pk] -> per-chunk S column offset (global)
        self.s_width = []                # [pk] -> per-chunk S width
        self.s_total = 0
        self.g_cols = []                 # [pk] total gather columns
        self.g_lo_cols = []              # [pk] lo gather columns
        self.idx_off = []                # [2*pk+ps] slot-stream offset
        self.idx_total = 0
        idx_cum = 0
        for pk in range(cfg.NPACK):
            self.chunks.append([[], []])
            self.s_off.append([])
            self.s_width.append([])
            g_col = 0
            for ps in range(2):
                n_slots = int(W[pk, :, ps].sum())
                n_pad = ((n_slots + P - 1) // P) * P
                bounds = np.cumsum(np.concatenate([[0], W[pk, :, ps]]))
                bounds[-1] = n_pad
                for c in range(n_pad // P):
                    s0, s1 = c * P, (c + 1) * P
                    w0 = min(int(np.searchsorted(bounds, s0, side="right")) - 1, 15)
                    w1 = min(int(np.searchsorted(bounds, s1 - 1, side="right")) - 1, 15)
                    col_off = w0 * cfg.WIN
                    width = min((w1 - w0 + 1) * cfg.WIN, 512 - col_off)
                    self.chunks[pk][ps].append((col_off, width, g_col))
                    self.s_off[pk].append(self.s_total)
                    self.s_width[pk].append(width)
                    self.s_total += width
                    g_col += 1
                self.idx_off.append(idx_cum)
                idx_cum += n_pad
                if ps == 0:
                    self.g_lo_cols.append(n_pad // P)
            self.g_cols.append(g_col)
        self.idx_total = idx_cum


def preprocess(cfg: Cfg, features, edge_row, edge_col, W_in, Ws, c, W_out):
    """Host-side preprocessing -> (in_maps, sched, perm)."""
    import heapq

    N, R, nc_ = cfg.N, cfg.R, cfg.n_cores
    f32 = np.float32
    deg = np.bincount(edge_row, minlength=N)
    deg_inv = (1.0 / np.maximum(deg, 1)).astype(f32)

    owner = edge_row // R
    is_hi = (edge_col // R) >= 5         # source in hi half of table

    n_win = 16 * cfg.NPACK
    n_full_win = 16 * (cfg.NPACK - 1)
    tail = R - 512 * (cfg.NPACK - 1)

    dest_of = np.empty((nc_, R), np.int64)    # local id -> orig local dest
    local_of = np.empty((nc_, R), np.int64)   # orig local dest -> local id
    load = np.zeros((nc_, cfg.NPACK, 16, 2), np.int64)
    dev_edges = []

    for d in range(nc_):
        m = owner == d
        erow = edge_row[m] - d * R
        dev_edges.append((erow, edge_col[m], is_hi[m]))
        dlo = np.bincount(erow[~is_hi[m]], minlength=R)
        dhi = np.bincount(erow[is_hi[m]], minlength=R)
        tot = dlo + dhi
        order = np.argsort(-tot, kind="stable")
        tail_d = order[-tail:] if tail > 0 else np.array([], np.int64)
        main_d = order[:len(order) - tail]
        wd = [[] for _ in range(n_win)]
        if n_full_win > 0:
            heap = [(0, w) for w in range(n_full_win)]
            heapq.heapify(heap)
            counts = [0] * n_full_win
            for o in main_d:
                while True:
                    ld, w = heapq.heappop(heap)
                    if counts[w] < cfg.WIN:
                        break
                wd[w].append(int(o))
                counts[w] += 1
                if counts[w] < cfg.WIN:
                    heapq.heappush(heap, (ld + int(tot[o]), w))
        for i, o in enumerate(tail_d):
            wd[n_full_win + i // cfg.WIN].append(int(o))
        # rank-sort full-pack windows by load desc within each pack
        for pk in range(cfg.NPACK - 1):
            ws = wd[16 * pk:16 * (pk + 1)]
            key = [-sum(int(tot[o]) for o in w) for w in ws]
            wd[16 * pk:16 * (pk + 1)] = [ws[i] for i in np.argsort(key, kind="stable")]
        for w in range(n_win):
            pk, wi = divmod(w, 16)
            for j, o in enumerate(wd[w]):
                li = 512 * pk + cfg.WIN * wi + j
                local_of[d, o] = li
                dest_of[d, li] = o
            load[d, pk, wi, 0] = int(dlo[wd[w]].sum()) if wd[w] else 0
            load[d, pk, wi, 1] = int(dhi[wd[w]].sum()) if wd[w] else 0

    sch = Sched(cfg, load.max(axis=0))
    W = sch.W

    pos = np.empty(N, np.int64)
    for d in range(nc_):
        pos[d * R + np.arange(R)] = d * R + local_of[d]

    # window slot-stream bases (uniform across devices)
    win_base = np.zeros((cfg.NPACK, 16, 2), np.int64)
    for pk in range(cfg.NPACK):
        for ps in range(2):
            win_base[pk, :, ps] = (sch.idx_off[2 * pk + ps] +
                                   np.concatenate([[0], np.cumsum(W[pk, :, ps])[:-1]]))
    # per-(pass-chunk) metadata as arrays for vectorized S fill
    ch_base = []   # [pk][ps] global chunk index of first chunk
    g = 0
    for pk in range(cfg.NPACK):
        ch_base.append([g, g + len(sch.chunks[pk][0])])
        g += len(sch.chunks[pk][0]) + len(sch.chunks[pk][1])
    all_co = np.array([co for pk in range(cfg.NPACK) for ps in range(2)
                       for (co, cw, gc) in sch.chunks[pk][ps]], np.int64)
    all_soff = np.array([o for pk in range(cfg.NPACK) for o in sch.s_off[pk]],
                        np.int64)

    # shared weights
    NFP = cfg.NFP
    W_aug = np.zeros((NFP, cfg.NHID), f32)
    W_aug[:cfg.NFEAT] = (1.0 - cfg.GAMMA) * W_in
    W_aug[cfg.NFEAT] = cfg.GAMMA * np.maximum(c, 0.0)
    nk = NFP // P
    W_dram = np.empty((P, nk * P), cfg.np_dt)
    for k in range(nk):
        W_dram[:, k * P:(k + 1) * P] = W_aug[k * P:(k + 1) * P]
    Ws_dram = np.empty((P, cfg.NLAYERS * P), cfg.np_dt)
    for l in range(cfg.NLAYERS):
        Ws_dram[:, l * P:(l + 1) * P] = Ws[l]
    Wout_dram = np.ascontiguousarray(W_out).astype(cfg.np_dt)

    in_maps = []
    for d in range(nc_):
        erow, ecol, ehi = dev_edges[d]
        li = local_of[d][erow]
        ps = ehi.astype(np.int64)
        key = li * 2 + ps
        order = np.argsort(key, kind="stable")
        key_s = key[order]
        ecol_s = ecol[order]
        cnt = np.bincount(key_s, minlength=2 * R)
        goff = np.concatenate([[0], np.cumsum(cnt)])
        idx_in_grp = np.arange(len(key_s)) - goff[key_s]
        # dest offset within its window's stream (per pass)
        cnt2 = cnt.reshape(R, 2)
        cnt_pad = np.zeros((cfg.NPACK * 512, 2), np.int64)
        cnt_pad[:R] = cnt2
        cw = cnt_pad.reshape(cfg.NPACK * 16, cfg.WIN, 2)
        dest_off = (np.cumsum(cw, axis=1) - cw).reshape(cfg.NPACK * 512, 2)
        li_s = key_s // 2
        ps_s = key_s % 2
        pk_s = li_s // 512
        wi_s = (li_s % 512) // cfg.WIN
        spos = (win_base[pk_s, wi_s, ps_s] + dest_off[li_s, ps_s] + idx_in_grp)
        # gather index values
        pv = pos[ecol_s]
        pv = np.where(ps_s == 1, pv - cfg.T_SPLIT, pv)
        idx_vals = np.zeros(sch.idx_total, np.int16)
        idx_vals[spos] = pv.astype(np.int16)
        # S fill: chunk of each slot & column within chunk
        seg_off = np.array(sch.idx_off + [sch.idx_total], np.int64)
        seg_id = 2 * pk_s + ps_s
        s_rel = spos - seg_off[seg_id]
        cch = np.array([ch_base[pk][ps] for pk in range(cfg.NPACK)
                        for ps in range(2)], np.int64)[seg_id] + s_rel // P
        srow = s_rel % P
        col_in_pack = li_s % 512
        scol = all_soff[cch] + (col_in_pack - all_co[cch])
        s_data = np.zeros((P, sch.s_total), cfg.np_dt)
        s_data[srow, scol] = deg_inv[d * R + dest_of[d][li_s]]
        # wrap idx into [16, total/16] segments then replicate to 128
        idx_t = np.zeros((16, sch.idx_total // 16), np.int16)
        for gi in range(2 * cfg.NPACK):
            b, e = seg_off[gi], seg_off[gi + 1]
            if e > b:
                idx_t[:, b // 16:e // 16] = idx_vals[b:e].reshape(-1, 16).T
        idx_t = np.tile(idx_t, (8, 1))

        gids = d * R + dest_of[d]
        featT = np.zeros((NFP, R), cfg.np_dt)
        featT[:cfg.NFEAT] = features[gids].T
        featT[cfg.NFEAT] = 1.0

        in_maps.append(dict(
            featT=featT, idx_all=np.ascontiguousarray(idx_t),
            s_all=s_data, w_proj=W_dram, w_hid=Ws_dram, w_out=Wout_dram,
        ))

    perm = np.concatenate([d * R + dest_of[d] for d in range(nc_)])
    return in_maps, sch, perm


def build_program(cfg: Cfg, sch: Sched, enable_asserts=False, rep=1, no_coll=False):
    import os
    skip = set(os.environ.get("GNN_SKIP", "").split(","))
    nc = bacc.Bacc("TRN2", target_bir_lowering=False, debug=False,
                   enable_asserts=enable_asserts,
                   num_devices=1 if no_coll else cfg.n_cores,
                   num_swdge_queues=4)
    DT = cfg.my_dt
    f32 = mybir.dt.float32
    R, NQ, NPACK, NFP = cfg.R, cfg.NQ, cfg.NPACK, cfg.NFP
    AFT = mybir.ActivationFunctionType
    ALU = mybir.AluOpType
    AX = mybir.AxisListType
    rg = [list(range(cfg.n_cores))]
    nk = NFP // P
    nc._gq = 0

    featT = nc.dram_tensor("featT", [NFP, R], DT, kind="ExternalInput").ap()
    idx_all = nc.dram_tensor("idx_all", [P, sch.idx_total // 16],
                             mybir.dt.int16, kind="ExternalInput").ap()
    s_all = nc.dram_tensor("s_all", [P, sch.s_total], DT,
                           kind="ExternalInput").ap()
    w_proj = nc.dram_tensor("w_proj", [P, nk * P], DT,
                            kind="ExternalInput").ap()
    w_hid = nc.dram_tensor("w_hid", [P, cfg.NLAYERS * P], DT,
                           kind="ExternalInput").ap()
    w_out = nc.dram_tensor("w_out", [P, cfg.NCLASS], DT,
                           kind="ExternalInput").ap()
    out = nc.dram_tensor("out", [R, cfg.NCLASS], f32,
                         kind="ExternalOutput").ap()

    INV08 = float(np.float32(1.0 / (np.float64(cfg.X2) - cfg.X1 + 1e-8)))
    B_RELU = float(np.float32(-cfg.X1 * INV08))
    E1 = float(1.0 + np.exp(-cfg.C_ACT))

    with tile.TileContext(nc) as tc:
        with tc.tile_pool(name="persist", bufs=1) as persist, \
             tc.tile_pool(name="dram", bufs=1, space="DRAM") as dram:
            # ---- persistent tiles ----
            idx_sb = persist.tile([P, sch.idx_total // 16], mybir.dt.int16)
            nc.sync.dma_start(idx_sb[:], idx_all[:])
            x0_sb = persist.tile([P, NQ * P], f32)
            wh_sb = persist.tile([P, cfg.NLAYERS * P], DT)
            nc.sync.dma_start(wh_sb[:], w_hid[:])
            wo_sb = persist.tile([P, cfg.NCLASS], DT)
            nc.sync.dma_start(wo_sb[:], w_out[:])
            wp_sb = persist.tile([P, nk * P], DT)
            nc.sync.dma_start(wp_sb[:], w_proj[:])
            zero1 = persist.tile([1, P], DT)
            nc.vector.memset(zero1[:], 0.0)
            zero512 = persist.tile([1, 512], DT)
            nc.vector.memset(zero512[:], 0.0)
            ones1 = persist.tile([1, P], f32)
            nc.vector.memset(ones1[:], 1.0)
            b_relu = persist.tile([P, 1], f32)
            nc.vector.memset(b_relu[:], B_RELU)
            idn = persist.tile([P, P], f32)
            make_identity(nc, idn[:])
            rmax = persist.tile([P, 1], f32)
            rmin = persist.tile([P, 1], f32)
            mm_sb = persist.tile([P, 2], f32)
            mm_red = persist.tile([1, 2], f32)
            mm_back = persist.tile([1, 2], f32)
            sfac = persist.tile([P, 1], f32)
            bfac = persist.tile([P, 1], f32)

            NIT = cfg.NLAYERS * rep
            x_full = [dram.tile([cfg.N, cfg.NHID], DT, addr_space="Shared",
                                name=f"x_full{i}") for i in range(NIT)]
            bounce = [dram.tile([R, cfg.NHID], DT, name=f"bounce{i}")
                      for i in range(NIT)]
            mm_in = dram.tile([1, 2], f32)
            mm_out = dram.tile([1, 2], f32, addr_space="Shared")

            # ================= projection phase =================
            with tc.tile_pool(name="strips", bufs=1) as strip_pool, \
                 tc.tile_pool(name="pwork", bufs=2) as pwork, \
                 tc.tile_pool(name="pps", bufs=2, space="PSUM") as pps_pool:
                strips = []
                for k in range(nk):
                    st = strip_pool.tile([P, R], DT, name=f"strip{k}",
                                         tag=f"strip{k}")
                    nc.sync.dma_start(st[:], featT[k * P:(k + 1) * P, :])
                    strips.append(st)
                for q in range(NQ):
                    r0 = q * P
                    w = min(P, R - r0)
                    h0ps = pps_pool.tile([P, P], f32, name="h0ps", tag="h0ps")
                    for k in range(nk):
                        nc.tensor.matmul(h0ps[:w, :], lhsT=strips[k][:, r0:r0 + w],
                                         rhs=wp_sb[:, k * P:(k + 1) * P],
                                         start=(k == 0), stop=(k == nk - 1))
                    nc.vector.tensor_copy(x0_sb[:w, q * P:(q + 1) * P],
                                          h0ps[:w, :])
                    qmax = pwork.tile([P, 1], f32, name="qmax", tag="qmax")
                    qmin = pwork.tile([P, 1], f32, name="qmin", tag="qmin")
                    nc.vector.tensor_reduce(qmax[:w], h0ps[:w, :], axis=AX.X,
                                            op=ALU.max)
                    nc.vector.tensor_reduce(qmin[:w], h0ps[:w, :], axis=AX.X,
                                            op=ALU.min)
                    if q == 0:
                        nc.vector.tensor_copy(rmax[:], qmax[:])
                        nc.vector.tensor_copy(rmin[:], qmin[:])
                    else:
                        nc.vector.tensor_tensor(rmax[:w], rmax[:w], qmax[:w],
                                                op=ALU.max)
                        nc.vector.tensor_tensor(rmin[:w], rmin[:w], qmin[:w],
                                                op=ALU.min)
                nc.vector.tensor_copy(mm_sb[:, 0:1], rmax[:])
                nc.vector.tensor_scalar(mm_sb[:, 1:2], rmin[:], -1.0, None,
                                        ALU.mult)
                nc.gpsimd.tensor_reduce(mm_red[:], mm_sb[:], axis=AX.C,
                                        op=ALU.max)
                nc.sync.dma_start(mm_in[:], mm_red[:])
                if no_coll:
                    nc.sync.dma_start(mm_back[:], mm_in[:])
                else:
                    nc.gpsimd.collective_compute(
                        "AllReduce", ALU.max, ins=[mm_in.opt()],
                        outs=[mm_out.opt()], replica_groups=rg)
                    nc.sync.dma_start(mm_back[:], mm_out[:])
                bc_ps = pps_pool.tile([P, 2], f32, name="bc_ps", tag="h0ps")
                nc.tensor.matmul(bc_ps[:], lhsT=ones1[:], rhs=mm_back[:],
                                 start=True, stop=True)
                bcast = pwork.tile([P, 2], f32, name="bcast", tag="qmin")
                nc.vector.tensor_copy(bcast[:], bc_ps[:])
                sden = pwork.tile([P, 1], f32, name="sden", tag="qmax")
                nc.vector.tensor_tensor(sden[:], bcast[:, 0:1], bcast[:, 1:2],
                                        op=ALU.add)
                nc.vector.tensor_scalar(sden[:], sden[:], 1e-8, None, ALU.add)
                nc.vector.reciprocal(sfac[:], sden[:])
                nc.vector.tensor_tensor(bfac[:], bcast[:, 1:2], sfac[:],
                                        op=ALU.mult)
                for q in range(NQ):
                    r0 = q * P
                    w = min(P, R - r0)
                    sl = slice(q * P, (q + 1) * P)
                    nc.vector.tensor_scalar(x0_sb[:w, sl], x0_sb[:w, sl],
                                            sfac[:w, :], bfac[:w, :],
                                            ALU.mult, ALU.add)
                    xq = pwork.tile([P, P], DT, name="xq", tag="xq")
                    nc.scalar.activation(xq[:w, :], x0_sb[:w, sl], AFT.Copy)
                    nc.sync.dma_start(bounce[0][r0:r0 + w, :], xq[:w, :])
            if no_coll:
                nc.sync.dma_start(x_full[0][:R, :], bounce[0][:])
            else:
                nc.gpsimd.collective_compute(
                    "AllGather", ALU.bypass, ins=[bounce[0].opt()],
                    outs=[x_full[0].opt()], replica_groups=rg)

            # ================= conv layers =================
            with tc.tile_pool(name="gpool", bufs=2) as gpool, \
                 tc.tile_pool(name="spool", bufs=2) as spool, \
                 tc.tile_pool(name="lwork", bufs=3) as work, \
                 tc.tile_pool(name="xnp", bufs=1) as xnp, \
                 tc.tile_pool(name="pack_ps", bufs=2, space="PSUM") as pack_ps, \
                 tc.tile_pool(name="z2_ps", bufs=2, space="PSUM") as z2_ps, \
                 tc.tile_pool(name="lg_ps", bufs=2, space="PSUM") as lg_ps:
                for l in range(cfg.NLAYERS * rep):
                    li = l % cfg.NLAYERS
                    last = l == cfg.NLAYERS * rep - 1
                    beta = min(0.5, (li + 1) / cfg.NLAYERS * 0.5)
                    c1 = float((1.0 - beta) * E1)
                    tbl = x_full[l]
                    x0b = xnp.tile([P, NQ * P], f32, name=f"x0b{l}", tag="x0b")
                    for q in range(NQ):
                        w = min(P, R - q * P)
                        sl = slice(q * P, (q + 1) * P)
                        nc.vector.tensor_scalar(x0b[:w, sl], x0_sb[:w, sl],
                                                float(beta), None, ALU.mult)
                    if not last:
                        xn = xnp.tile([P, NQ * P], DT, name=f"xn{l}", tag="xn")
                    for pk in range(NPACK):
                        ncol = sch.g_cols[pk]
                        nlo = sch.g_lo_cols[pk]
                        gt = gpool.tile([P, max(ncol, 1), P], DT,
                                        name=f"g{l}_{pk}", tag="g")
                        i0 = sch.idx_off[2 * pk] // 16
                        n_lo = nlo * P
                        n_hi = (ncol - nlo) * P
                        if n_lo and "gather" not in skip:
                            nc.gpsimd.dma_gather(
                                out_ap=gt[:, :nlo, :],
                                in_ap=tbl[:cfg.T_SPLIT, :],
                                idxs_ap=idx_sb[:, i0:i0 + n_lo // 16],
                                num_idxs=n_lo, num_idxs_reg=n_lo,
                                elem_size=cfg.NHID, single_packet=False,
                                queue_num=nc._gq % 4)
                            nc._gq += 1
                        if n_hi:
                            i1 = sch.idx_off[2 * pk + 1] // 16
                            nc.gpsimd.dma_gather(
                                out_ap=gt[:, nlo:, :],
                                in_ap=tbl[cfg.T_SPLIT:, :],
                                idxs_ap=idx_sb[:, i1:i1 + n_hi // 16],
                                num_idxs=n_hi, num_idxs_reg=n_hi,
                                elem_size=cfg.NHID, single_packet=False,
                                queue_num=nc._gq % 4)
                            nc._gq += 1
                        so = sch.s_off[pk][0] if sch.s_off[pk] else 0
                        s_w = sum(sch.s_width[pk])
                        if s_w and "sload" not in skip:
                            s_sb = spool.tile([P, s_w], DT, name=f"s{l}_{pk}",
                                              tag="s")
                            nc.sync.dma_start(s_sb[:], s_all[:, so:so + s_w])
                        elif s_w:
                            s_sb = spool.tile([P, s_w], DT, name=f"s{l}_{pk}",
                                              tag="s")
                        pps = pack_ps.tile([P, 512], f32, name=f"pps{l}_{pk}",
                                           tag="pps")
                        n_ch = (len(sch.chunks[pk][0]) + len(sch.chunks[pk][1])
                                if "chunks" not in skip else 0)
                        nc.tensor.matmul(pps[:], lhsT=zero1[:], rhs=zero512[:],
                                         start=True, stop=(n_ch == 0),
                                         skip_group_check=True)
                        ci = 0
                        chunk_sched = sch.chunks if "chunks" not in skip else [[[], []]] * cfg.NPACK
                        for ps in range(2):
                            for (co, cw, gc) in chunk_sched[pk][ps]:
                                s0 = sch.s_off[pk][ci] - so
                                nc.tensor.matmul(
                                    pps[:, co:co + cw], lhsT=gt[:, gc, :],
                                    rhs=s_sb[:, s0:s0 + cw],
                                    start=False, stop=(ci == n_ch - 1),
                                    skip_group_check=True)
                                ci += 1
                        sT = work.tile([P, 512], DT, name="sT", tag="sT")
                        nc.vector.tensor_copy(sT[:], pps[:])
                        for qq in range(4):
                            q = 4 * pk + qq
                            r0 = q * P
                            if r0 >= R:
                                break
                            w = min(P, R - r0)
                            sl = slice(q * P, (q + 1) * P)
                            z2 = z2_ps.tile([P, P], f32, name="z2", tag="z2")
                            nc.tensor.matmul(z2[:w, :],
                                             lhsT=sT[:, qq * P:qq * P + w],
                                             rhs=wh_sb[:, li * P:(li + 1) * P],
                                             start=True, stop=True)
                            a1 = work.tile([P, P], f32, name="a1", tag="a1")
                            nc.scalar.activation(a1[:w], z2[:w, :], AFT.Relu,
                                                 bias=b_relu[:w], scale=INV08)
                            nc.vector.tensor_scalar(a1[:w], a1[:w], 1.0, c1,
                                                    ALU.min, ALU.mult)
                            a3 = work.tile([P, P], f32, name="a3", tag="a3")
                            nc.scalar.activation(a3[:w], a1[:w], AFT.Sigmoid,
                                                 scale=float(-1.0 / c1))
                            nc.vector.tensor_tensor(a1[:w], a1[:w], a3[:w],
                                                    op=ALU.mult)
                            if not last:
                                nc.vector.tensor_tensor(xn[:w, sl], a1[:w],
                                                        x0b[:w, sl], op=ALU.add)
                                nc.sync.dma_start(bounce[l + 1][r0:r0 + w, :],
                                                  xn[:w, sl])
                            else:
                                x4 = work.tile([P, P], f32, name="x4", tag="a3")
                                nc.vector.tensor_tensor(x4[:w], a1[:w],
                                                        x0b[:w, sl], op=ALU.add)
                                tps = z2_ps.tile([P, P], f32, name="tps",
                                                 tag="z2")
                                nc.tensor.transpose(tps[:, :w], x4[:w],
                                                    identity=idn[:w, :w])
                                x4T = work.tile([P, P], DT, name="x4T",
                                                tag="a1")
                                nc.vector.tensor_copy(x4T[:, :w], tps[:, :w])
                                lg = lg_ps.tile([P, cfg.NCLASS], f32,
                                                name="lg", tag="lg")
                                nc.tensor.matmul(lg[:w, :], lhsT=x4T[:, :w],
                                                 rhs=wo_sb[:], start=True,
                                                 stop=True)
                                mx = work.tile([P, 1], f32, name="mx", tag="mx")
                                nc.vector.tensor_reduce(mx[:w], lg[:w, :],
                                                        axis=AX.X, op=ALU.max)
                                nmx = work.tile([P, 1], f32, name="nmx",
                                                tag="nmx")
                                nc.vector.tensor_scalar(nmx[:w], mx[:w], -1.0,
                                                        None, ALU.mult)
                                ex = work.tile([P, cfg.NCLASS], f32, name="ex",
                                               tag="ex")
                                sume = work.tile([P, 1], f32, name="sume",
                                                 tag="sume")
                                nc.scalar.activation(ex[:w], lg[:w, :], AFT.Exp,
                                                     bias=nmx[:w],
                                                     accum_out=sume[:w])
                                lse = work.tile([P, 1], f32, name="lse",
                                                tag="lse")
                                nc.scalar.activation(lse[:w], sume[:w], AFT.Ln)
                                nc.vector.tensor_tensor(lse[:w], lse[:w],
                                                        nmx[:w], op=ALU.subtract)
                                res = work.tile([P, cfg.NCLASS], f32,
                                                name="res", tag="ex")
                                nc.vector.tensor_scalar(res[:w], lg[:w, :],
                                                        lse[:w], None,
                                                        ALU.subtract)
                                nc.sync.dma_start(out[r0:r0 + w, :], res[:w])
                    if not last:
                        if no_coll:
                            nc.sync.dma_start(x_full[l + 1][:R, :],
                                              bounce[l + 1][:])
                        else:
                            nc.gpsimd.collective_compute(
                                "AllGather", ALU.bypass,
                                ins=[bounce[l + 1].opt()],
                                outs=[x_full[l + 1].opt()],
                                replica_groups=rg)
    nc.compile()
    return nc


def kernel(**inputs) -> np.ndarray:
    cfg = Cfg()
    features = np.asarray(inputs["features"], np.float32)
    edge_row = np.asarray(inputs["edge_row"], np.int64)
    edge_col = np.asarray(inputs["edge_col"], np.int64)
    W_in = np.asarray(inputs["W_in"], np.float32)
    Ws = np.asarray(inputs["Ws"], np.float32)
    c = np.asarray(inputs["c"], np.float32)
    W_out = np.asarray(inputs["W_out"], np.float32)

    in_maps, sch, perm = preprocess(cfg, features, edge_row, edge_col,
                                    W_in, Ws, c, W_out)
    nc = build_program(cfg, sch)

    import os
    from concourse import bass_utils
    res = bass_utils.run_bass_kernel_spmd(
        nc, in_maps, core_ids=list(range(cfg.n_cores)),
        trace=bool(os.environ.get("GNN_TRACE")))
    kernel.last_result = res
    out = np.empty((cfg.N, cfg.NCLASS), np.float32)
    for d in range(cfg.n_cores):
        out[perm[d * cfg.R:(d + 1) * cfg.R]] = res.results[d]["out"]
    return out



# revision 6
# speedup vs baseline: 2.0809x; 1.1829x over previous
"""Trainium2 Bass kernel for nn_NewActivationGNN (GNN message passing).

Self-contained: hardcodes problem shapes. Distributes across 8 NeuronCores:
nodes sharded by destination (graph parallel), per-layer AllGather of the
hidden-state table, per-edge gathers via the custom dma_gather instruction,
and segment-sum via per-chunk selection-matrix matmuls accumulated in PSUM
(feature-major packs, one PSUM bank per 512 destination columns).

SPMD: one program runs on all 8 cores; all per-device variation (indices,
selection matrices, features) is data. The compile-time chunk schedule is
made device-uniform by padding per-window slot budgets to the cross-device
maximum.
"""

import sys

for _p in ("/opt/trn_rl_repo", "/root/.axon_site/_ro/trn_rl_repo"):
    if _p not in sys.path:
        sys.path.insert(0, _p)

import math
from dataclasses import dataclass

import numpy as np

import concourse.bass as bass  # noqa: F401
import concourse.tile as tile
from concourse import bacc, mybir
from concourse.masks import make_identity

P = 128


@dataclass
class Cfg:
    N: int = 50000
    E: int = 800000
    NFEAT: int = 500
    NHID: int = 128
    NCLASS: int = 40
    NLAYERS: int = 4
    GAMMA: float = 0.3
    X1: float = 0.1
    X2: float = 0.9
    C_ACT: float = -1.0
    n_cores: int = 8
    WIN: int = 32            # dest columns per window
    dt: str = "float16"      # gather-table / matmul operand dtype

    @property
    def R(self):
        return self.N // self.n_cores

    @property
    def T_SPLIT(self):       # lo/hi split of the permuted table (4 devices)
        return 4 * self.R

    @property
    def NPACK(self):
        return math.ceil(self.R / 512)

    @property
    def NQ(self):
        return math.ceil(self.R / P)

    @property
    def NFP(self):
        return ((self.NFEAT + 1 + P - 1) // P) * P

    @property
    def np_dt(self):
        return np.float16 if self.dt == "float16" else np.float32

    @property
    def my_dt(self):
        return mybir.dt.float16 if self.dt == "float16" else mybir.dt.float32


class Sched:
    """Device-uniform compile-time schedule derived from cross-device-max
    window slot budgets W[pack, win, pass]."""

    def __init__(self, cfg: Cfg, W: np.ndarray):
        self.W = W                       # [NPACK, 16, 2] multiples not needed
        self.chunks = []                 # [pk][ps] -> list of (col_off, width, g_col)
        self.s_off = []                  # [pk] -> per-chunk S column offset (global)
        self.s_width = []                # [pk] -> per-chunk S width
        self.s_total = 0
        self.g_cols = []                 # [pk] total gather columns
        self.g_lo_cols = []              # [pk] lo gather columns
        self.idx_off = []                # [2*pk+ps] slot-stream offset
        self.idx_total = 0
        idx_cum = 0
        for pk in range(cfg.NPACK):
            self.chunks.append([[], []])
            self.s_off.append([])
            self.s_width.append([])
            g_col = 0
            for ps in range(2):
                n_slots = int(W[pk, :, ps].sum())
                n_pad = ((n_slots + P - 1) // P) * P
                bounds = np.cumsum(np.concatenate([[0], W[pk, :, ps]]))
                bounds[-1] = n_pad
                for c in range(n_pad // P):
                    s0, s1 = c * P, (c + 1) * P
                    w0 = min(int(np.searchsorted(bounds, s0, side="right")) - 1, 15)
                    w1 = min(int(np.searchsorted(bounds, s1 - 1, side="right")) - 1, 15)
                    col_off = w0 * cfg.WIN
                    width = min((w1 - w0 + 1) * cfg.WIN, 512 - col_off)
                    self.chunks[pk][ps].append((col_off, width, g_col))
                    self.s_off[pk].append(self.s_total)
                    self.s_width[pk].append(width)
                    self.s_total += width
                    g_col += 1
                self.idx_off.append(idx_cum)
                idx_cum += n_pad
                if ps == 0:
                    self.g_lo_cols.append(n_pad // P)
            self.g_cols.append(g_col)
        self.idx_total = idx_cum


def preprocess(cfg: Cfg, features, edge_row, edge_col, W_in, Ws, c, W_out):
    """Host-side preprocessing -> (in_maps, sched, perm)."""
    import heapq

    N, R, nc_ = cfg.N, cfg.R, cfg.n_cores
    f32 = np.float32
    deg = np.bincount(edge_row, minlength=N)
    deg_inv = (1.0 / np.maximum(deg, 1)).astype(f32)

    owner = edge_row // R
    is_hi = (edge_col // R) >= cfg.T_SPLIT // R   # source in hi half of table

    n_win = 16 * cfg.NPACK
    n_full_win = 16 * (cfg.NPACK - 1)
    tail = R - 512 * (cfg.NPACK - 1)

    dest_of = np.empty((nc_, R), np.int64)    # local id -> orig local dest
    local_of = np.empty((nc_, R), np.int64)   # orig local dest -> local id
    load = np.zeros((nc_, cfg.NPACK, 16, 2), np.int64)
    dev_edges = []

    for d in range(nc_):
        m = owner == d
        erow = edge_row[m] - d * R
        dev_edges.append((erow, edge_col[m], is_hi[m]))
        dlo = np.bincount(erow[~is_hi[m]], minlength=R)
        dhi = np.bincount(erow[is_hi[m]], minlength=R)
        tot = dlo + dhi
        order = np.argsort(-tot, kind="stable")
        tail_d = order[-tail:] if tail > 0 else np.array([], np.int64)
        main_d = order[:len(order) - tail]
        wd = [[] for _ in range(n_win)]
        if n_full_win > 0:
            heap = [(0, w) for w in range(n_full_win)]
            heapq.heapify(heap)
            counts = [0] * n_full_win
            for o in main_d:
                while True:
                    ld, w = heapq.heappop(heap)
                    if counts[w] < cfg.WIN:
                        break
                wd[w].append(int(o))
                counts[w] += 1
                if counts[w] < cfg.WIN:
                    heapq.heappush(heap, (ld + int(tot[o]), w))
        for i, o in enumerate(tail_d):
            wd[n_full_win + i // cfg.WIN].append(int(o))
        # rank-sort full-pack windows by load desc within each pack
        for pk in range(cfg.NPACK - 1):
            ws = wd[16 * pk:16 * (pk + 1)]
            key = [-sum(int(tot[o]) for o in w) for w in ws]
            wd[16 * pk:16 * (pk + 1)] = [ws[i] for i in np.argsort(key, kind="stable")]
        for w in range(n_win):
            pk, wi = divmod(w, 16)
            for j, o in enumerate(wd[w]):
                li = 512 * pk + cfg.WIN * wi + j
                local_of[d, o] = li
                dest_of[d, li] = o
            load[d, pk, wi, 0] = int(dlo[wd[w]].sum()) if wd[w] else 0
            load[d, pk, wi, 1] = int(dhi[wd[w]].sum()) if wd[w] else 0

    sch = Sched(cfg, load.max(axis=0))
    W = sch.W

    pos = np.empty(N, np.int64)
    for d in range(nc_):
        pos[d * R + np.arange(R)] = d * R + local_of[d]

    # window slot-stream bases (uniform across devices)
    win_base = np.zeros((cfg.NPACK, 16, 2), np.int64)
    for pk in range(cfg.NPACK):
        for ps in range(2):
            win_base[pk, :, ps] = (sch.idx_off[2 * pk + ps] +
                                   np.concatenate([[0], np.cumsum(W[pk, :, ps])[:-1]]))
    # per-(pass-chunk) metadata as arrays for vectorized S fill
    ch_base = []   # [pk][ps] global chunk index of first chunk
    g = 0
    for pk in range(cfg.NPACK):
        ch_base.append([g, g + len(sch.chunks[pk][0])])
        g += len(sch.chunks[pk][0]) + len(sch.chunks[pk][1])
    all_co = np.array([co for pk in range(cfg.NPACK) for ps in range(2)
                       for (co, cw, gc) in sch.chunks[pk][ps]], np.int64)
    all_soff = np.array([o for pk in range(cfg.NPACK) for o in sch.s_off[pk]],
                        np.int64)

    # shared weights
    NFP = cfg.NFP
    W_aug = np.zeros((NFP, cfg.NHID), f32)
    W_aug[:cfg.NFEAT] = (1.0 - cfg.GAMMA) * W_in
    W_aug[cfg.NFEAT] = cfg.GAMMA * np.maximum(c, 0.0)
    nk = NFP // P
    W_dram = np.empty((P, nk * P), cfg.np_dt)
    for k in range(nk):
        W_dram[:, k * P:(k + 1) * P] = W_aug[k * P:(k + 1) * P]
    Ws_dram = np.empty((P, cfg.NLAYERS * P), cfg.np_dt)
    for l in range(cfg.NLAYERS):
        Ws_dram[:, l * P:(l + 1) * P] = Ws[l]
    Wout_dram = np.ascontiguousarray(W_out).astype(cfg.np_dt)

    in_maps = []
    for d in range(nc_):
        erow, ecol, ehi = dev_edges[d]
        li = local_of[d][erow]
        ps = ehi.astype(np.int64)
        key = li * 2 + ps
        order = np.argsort(key, kind="stable")
        key_s = key[order]
        ecol_s = ecol[order]
        cnt = np.bincount(key_s, minlength=2 * R)
        goff = np.concatenate([[0], np.cumsum(cnt)])
        idx_in_grp = np.arange(len(key_s)) - goff[key_s]
        # dest offset within its window's stream (per pass)
        cnt2 = cnt.reshape(R, 2)
        cnt_pad = np.zeros((cfg.NPACK * 512, 2), np.int64)
        cnt_pad[:R] = cnt2
        cw = cnt_pad.reshape(cfg.NPACK * 16, cfg.WIN, 2)
        dest_off = (np.cumsum(cw, axis=1) - cw).reshape(cfg.NPACK * 512, 2)
        li_s = key_s // 2
        ps_s = key_s % 2
        pk_s = li_s // 512
        wi_s = (li_s % 512) // cfg.WIN
        spos = (win_base[pk_s, wi_s, ps_s] + dest_off[li_s, ps_s] + idx_in_grp)
        # gather index values
        pv = pos[ecol_s]
        pv = np.where(ps_s == 1, pv - cfg.T_SPLIT, pv)
        idx_vals = np.zeros(sch.idx_total, np.int16)
        idx_vals[spos] = pv.astype(np.int16)
        # S fill: chunk of each slot & column within chunk
        seg_off = np.array(sch.idx_off + [sch.idx_total], np.int64)
        seg_id = 2 * pk_s + ps_s
        s_rel = spos - seg_off[seg_id]
        cch = np.array([ch_base[pk][ps] for pk in range(cfg.NPACK)
                        for ps in range(2)], np.int64)[seg_id] + s_rel // P
        srow = s_rel % P
        col_in_pack = li_s % 512
        scol = all_soff[cch] + (col_in_pack - all_co[cch])
        s_data = np.zeros((P, sch.s_total), cfg.np_dt)
        s_data[srow, scol] = deg_inv[d * R + dest_of[d][li_s]]
        # wrap idx into [16, total/16] segments then replicate to 128
        idx_t = np.zeros((16, sch.idx_total // 16), np.int16)
        for gi in range(2 * cfg.NPACK):
            b, e = seg_off[gi], seg_off[gi + 1]
            if e > b:
                idx_t[:, b // 16:e // 16] = idx_vals[b:e].reshape(-1, 16).T
        idx_t = np.tile(idx_t, (8, 1))

        gids = d * R + dest_of[d]
        featT = np.zeros((NFP, R), cfg.np_dt)
        featT[:cfg.NFEAT] = features[gids].T
        featT[cfg.NFEAT] = 1.0

        in_maps.append(dict(
            featT=featT, idx_all=np.ascontiguousarray(idx_t),
            s_all=s_data, w_proj=W_dram, w_hid=Ws_dram, w_out=Wout_dram,
        ))

    perm = np.concatenate([d * R + dest_of[d] for d in range(nc_)])
    return in_maps, sch, perm


def build_program(cfg: Cfg, sch: Sched, enable_asserts=False, rep=1, no_coll=False):
    import os
    skip = set(os.environ.get("GNN_SKIP", "").split(","))
    nc = bacc.Bacc("TRN2", target_bir_lowering=False, debug=False,
                   enable_asserts=enable_asserts,
                   num_devices=1 if no_coll else cfg.n_cores,
                   num_swdge_queues=4)
    DT = cfg.my_dt
    f32 = mybir.dt.float32
    R, NQ, NPACK, NFP = cfg.R, cfg.NQ, cfg.NPACK, cfg.NFP
    AFT = mybir.ActivationFunctionType
    ALU = mybir.AluOpType
    AX = mybir.AxisListType
    rg = [list(range(cfg.n_cores))]
    nk = NFP // P
    nc._gq = 0

    featT = nc.dram_tensor("featT", [NFP, R], DT, kind="ExternalInput").ap()
    idx_all = nc.dram_tensor("idx_all", [P, sch.idx_total // 16],
                             mybir.dt.int16, kind="ExternalInput").ap()
    s_all = nc.dram_tensor("s_all", [P, sch.s_total], DT,
                           kind="ExternalInput").ap()
    w_proj = nc.dram_tensor("w_proj", [P, nk * P], DT,
                            kind="ExternalInput").ap()
    w_hid = nc.dram_tensor("w_hid", [P, cfg.NLAYERS * P], DT,
                           kind="ExternalInput").ap()
    w_out = nc.dram_tensor("w_out", [P, cfg.NCLASS], DT,
                           kind="ExternalInput").ap()
    out = nc.dram_tensor("out", [R, cfg.NCLASS], f32,
                         kind="ExternalOutput").ap()

    INV08 = float(np.float32(1.0 / (np.float64(cfg.X2) - cfg.X1 + 1e-8)))
    B_RELU = float(np.float32(-cfg.X1 * INV08))
    E1 = float(1.0 + np.exp(-cfg.C_ACT))

    with tile.TileContext(nc) as tc:
        with tc.tile_pool(name="persist", bufs=1) as persist, \
             tc.tile_pool(name="dram", bufs=1, space="DRAM") as dram:
            # ---- persistent tiles ----
            idx_sb = persist.tile([P, sch.idx_total // 16], mybir.dt.int16)
            nc.sync.dma_start(idx_sb[:], idx_all[:])
            x0_sb = persist.tile([P, NQ * P], f32)
            wh_sb = persist.tile([P, cfg.NLAYERS * P], DT)
            nc.sync.dma_start(wh_sb[:], w_hid[:])
            wo_sb = persist.tile([P, cfg.NCLASS], DT)
            nc.sync.dma_start(wo_sb[:], w_out[:])
            wp_sb = persist.tile([P, nk * P], DT)
            nc.sync.dma_start(wp_sb[:], w_proj[:])
            zero1 = persist.tile([1, P], DT)
            nc.vector.memset(zero1[:], 0.0)
            zero512 = persist.tile([1, 512], DT)
            nc.vector.memset(zero512[:], 0.0)
            ones1 = persist.tile([1, P], f32)
            nc.vector.memset(ones1[:], 1.0)
            b_relu = persist.tile([P, 1], f32)
            nc.vector.memset(b_relu[:], B_RELU)
            idn = persist.tile([P, P], f32)
            make_identity(nc, idn[:])
            rmax = persist.tile([P, 1], f32)
            rmin = persist.tile([P, 1], f32)
            mm_sb = persist.tile([P, 2], f32)
            mm_red = persist.tile([1, 2], f32)
            mm_back = persist.tile([1, 2], f32)
            sfac = persist.tile([P, 1], f32)
            bfac = persist.tile([P, 1], f32)

            NIT = cfg.NLAYERS * rep
            x_full = [dram.tile([cfg.N, cfg.NHID], DT, addr_space="Shared",
                                name=f"x_full{i}") for i in range(NIT)]
            bounce = [dram.tile([R, cfg.NHID], DT, name=f"bounce{i}")
                      for i in range(NIT)]
            mm_in = dram.tile([1, 2], f32)
            mm_out = dram.tile([1, 2], f32, addr_space="Shared")

            # ================= projection phase =================
            with tc.tile_pool(name="strips", bufs=1) as strip_pool, \
                 tc.tile_pool(name="pwork", bufs=2) as pwork, \
                 tc.tile_pool(name="pps", bufs=2, space="PSUM") as pps_pool:
                strips = []
                for k in range(nk):
                    st = strip_pool.tile([P, R], DT, name=f"strip{k}",
                                         tag=f"strip{k}")
                    nc.sync.dma_start(st[:], featT[k * P:(k + 1) * P, :])
                    strips.append(st)
                for q in range(NQ):
                    r0 = q * P
                    w = min(P, R - r0)
                    h0ps = pps_pool.tile([P, P], f32, name="h0ps", tag="h0ps")
                    for k in range(nk):
                        nc.tensor.matmul(h0ps[:w, :], lhsT=strips[k][:, r0:r0 + w],
                                         rhs=wp_sb[:, k * P:(k + 1) * P],
                                         start=(k == 0), stop=(k == nk - 1))
                    nc.vector.tensor_copy(x0_sb[:w, q * P:(q + 1) * P],
                                          h0ps[:w, :])
                    qmax = pwork.tile([P, 1], f32, name="qmax", tag="qmax")
                    qmin = pwork.tile([P, 1], f32, name="qmin", tag="qmin")
                    nc.vector.tensor_reduce(qmax[:w], h0ps[:w, :], axis=AX.X,
                                            op=ALU.max)
                    nc.vector.tensor_reduce(qmin[:w], h0ps[:w, :], axis=AX.X,
                                            op=ALU.min)
                    if q == 0:
                        nc.vector.tensor_copy(rmax[:], qmax[:])
                        nc.vector.tensor_copy(rmin[:], qmin[:])
                    else:
                        nc.vector.tensor_tensor(rmax[:w], rmax[:w], qmax[:w],
                                                op=ALU.max)
                        nc.vector.tensor_tensor(rmin[:w], rmin[:w], qmin[:w],
                                                op=ALU.min)
                nc.vector.tensor_copy(mm_sb[:, 0:1], rmax[:])
                nc.vector.tensor_scalar(mm_sb[:, 1:2], rmin[:], -1.0, None,
                                        ALU.mult)
                nc.gpsimd.tensor_reduce(mm_red[:], mm_sb[:], axis=AX.C,
                                        op=ALU.max)
                nc.sync.dma_start(mm_in[:], mm_red[:])
                if no_coll:
                    nc.sync.dma_start(mm_back[:], mm_in[:])
                else:
                    nc.gpsimd.collective_compute(
                        "AllReduce", ALU.max, ins=[mm_in.opt()],
                        outs=[mm_out.opt()], replica_groups=rg)
                    nc.sync.dma_start(mm_back[:], mm_out[:])
                bc_ps = pps_pool.tile([P, 2], f32, name="bc_ps", tag="h0ps")
                nc.tensor.matmul(bc_ps[:], lhsT=ones1[:], rhs=mm_back[:],
                                 start=True, stop=True)
                bcast = pwork.tile([P, 2], f32, name="bcast", tag="qmin")
                nc.vector.tensor_copy(bcast[:], bc_ps[:])
                sden = pwork.tile([P, 1], f32, name="sden", tag="qmax")
                nc.vector.tensor_tensor(sden[:], bcast[:, 0:1], bcast[:, 1:2],
                                        op=ALU.add)
                nc.vector.tensor_scalar(sden[:], sden[:], 1e-8, None, ALU.add)
                nc.vector.reciprocal(sfac[:], sden[:])
                nc.vector.tensor_tensor(bfac[:], bcast[:, 1:2], sfac[:],
                                        op=ALU.mult)
                for q in range(NQ):
                    r0 = q * P
                    w = min(P, R - r0)
                    sl = slice(q * P, (q + 1) * P)
                    nc.vector.tensor_scalar(x0_sb[:w, sl], x0_sb[:w, sl],
                                            sfac[:w, :], bfac[:w, :],
                                            ALU.mult, ALU.add)
                    xq = pwork.tile([P, P], DT, name="xq", tag="xq")
                    nc.scalar.activation(xq[:w, :], x0_sb[:w, sl], AFT.Copy)
                    nc.sync.dma_start(bounce[0][r0:r0 + w, :], xq[:w, :])
            if no_coll:
                nc.sync.dma_start(x_full[0][:R, :], bounce[0][:])
            else:
                nc.gpsimd.collective_compute(
                    "AllGather", ALU.bypass, ins=[bounce[0].opt()],
                    outs=[x_full[0].opt()], replica_groups=rg)

            # ================= conv layers =================
            with tc.tile_pool(name="gpool", bufs=3) as gpool, \
                 tc.tile_pool(name="spool", bufs=3) as spool, \
                 tc.tile_pool(name="lwork", bufs=3) as work, \
                 tc.tile_pool(name="xnp", bufs=1) as xnp, \
                 tc.tile_pool(name="pack_ps", bufs=2, space="PSUM") as pack_ps, \
                 tc.tile_pool(name="z2_ps", bufs=2, space="PSUM") as z2_ps, \
                 tc.tile_pool(name="lg_ps", bufs=2, space="PSUM") as lg_ps:
                for l in range(cfg.NLAYERS * rep):
                    li = l % cfg.NLAYERS
                    last = l == cfg.NLAYERS * rep - 1
                    beta = min(0.5, (li + 1) / cfg.NLAYERS * 0.5)
                    c1 = float((1.0 - beta) * E1)
                    tbl = x_full[l]
                    x0b = xnp.tile([P, NQ * P], f32, name=f"x0b{l}", tag="x0b")
                    for q in range(NQ):
                        w = min(P, R - q * P)
                        sl = slice(q * P, (q + 1) * P)
                        nc.vector.tensor_scalar(x0b[:w, sl], x0_sb[:w, sl],
                                                float(beta), None, ALU.mult)
                    if not last:
                        xn = xnp.tile([P, NQ * P], DT, name=f"xn{l}", tag="xn")
                    for pk in range(NPACK):
                        ncol = sch.g_cols[pk]
                        nlo = sch.g_lo_cols[pk]
                        gt = gpool.tile([P, max(ncol, 1), P], DT,
                                        name=f"g{l}_{pk}", tag="g")
                        i0 = sch.idx_off[2 * pk] // 16
                        n_lo = nlo * P
                        n_hi = (ncol - nlo) * P
                        if n_lo and "gather" not in skip:
                            nc.gpsimd.dma_gather(
                                out_ap=gt[:, :nlo, :],
                                in_ap=tbl[:cfg.T_SPLIT, :],
                                idxs_ap=idx_sb[:, i0:i0 + n_lo // 16],
                                num_idxs=n_lo, num_idxs_reg=n_lo,
                                elem_size=cfg.NHID, single_packet=False,
                                queue_num=nc._gq % 4)
                            nc._gq += 1
                        if n_hi:
                            i1 = sch.idx_off[2 * pk + 1] // 16
                            nc.gpsimd.dma_gather(
                                out_ap=gt[:, nlo:, :],
                                in_ap=tbl[cfg.T_SPLIT:, :],
                                idxs_ap=idx_sb[:, i1:i1 + n_hi // 16],
                                num_idxs=n_hi, num_idxs_reg=n_hi,
                                elem_size=cfg.NHID, single_packet=False,
                                queue_num=nc._gq % 4)
                            nc._gq += 1
                        so = sch.s_off[pk][0] if sch.s_off[pk] else 0
                        s_w = sum(sch.s_width[pk])
                        if s_w and "sload" not in skip:
                            s_sb = spool.tile([P, s_w], DT, name=f"s{l}_{pk}",
                                              tag="s")
                            nc.sync.dma_start(s_sb[:], s_all[:, so:so + s_w])
                        elif s_w:
                            s_sb = spool.tile([P, s_w], DT, name=f"s{l}_{pk}",
                                              tag="s")
                        pps = pack_ps.tile([P, 512], f32, name=f"pps{l}_{pk}",
                                           tag="pps")
                        n_ch = (len(sch.chunks[pk][0]) + len(sch.chunks[pk][1])
                                if "chunks" not in skip else 0)
                        nc.tensor.matmul(pps[:], lhsT=zero1[:], rhs=zero512[:],
                                         start=True, stop=(n_ch == 0),
                                         skip_group_check=True)
                        ci = 0
                        chunk_sched = sch.chunks if "chunks" not in skip else [[[], []]] * cfg.NPACK
                        for ps in range(2):
                            for (co, cw, gc) in chunk_sched[pk][ps]:
                                s0 = sch.s_off[pk][ci] - so
                                nc.tensor.matmul(
                                    pps[:, co:co + cw], lhsT=gt[:, gc, :],
                                    rhs=s_sb[:, s0:s0 + cw],
                                    start=False, stop=(ci == n_ch - 1),
                                    skip_group_check=True)
                                ci += 1
                        sT = work.tile([P, 512], DT, name="sT", tag="sT")
                        nc.vector.tensor_copy(sT[:], pps[:])
                        for qq in range(4):
                            q = 4 * pk + qq
                            r0 = q * P
                            if r0 >= R:
                                break
                            w = min(P, R - r0)
                            sl = slice(q * P, (q + 1) * P)
                            z2 = z2_ps.tile([P, P], f32, name="z2", tag="z2")
                            nc.tensor.matmul(z2[:w, :],
                                             lhsT=sT[:, qq * P:qq * P + w],
                                             rhs=wh_sb[:, li * P:(li + 1) * P],
                                             start=True, stop=True)
                            a1 = work.tile([P, P], f32, name="a1", tag="a1")
                            nc.scalar.activation(a1[:w], z2[:w, :], AFT.Relu,
                                                 bias=b_relu[:w], scale=INV08)
                            nc.vector.tensor_scalar(a1[:w], a1[:w], 1.0, c1,
                                                    ALU.min, ALU.mult)
                            a3 = work.tile([P, P], f32, name="a3", tag="a3")
                            if not last:
                                nc.scalar.activation(a3[:w], a1[:w], AFT.Sigmoid,
                                                     scale=float(-1.0 / c1))
                            else:
                                # sigmoid(-a1/c1) = 1/(1+e^{a1/c1}); Exp keeps
                                # the last layer on one act-table set (exp+ln)
                                nc.scalar.activation(a3[:w], a1[:w], AFT.Exp,
                                                     scale=float(1.0 / c1))
                                nc.vector.tensor_scalar(a3[:w], a3[:w], 1.0,
                                                        None, ALU.add)
                                nc.vector.reciprocal(a3[:w], a3[:w])
                            nc.vector.tensor_tensor(a1[:w], a1[:w], a3[:w],
                                                    op=ALU.mult)
                            if not last:
                                nc.vector.tensor_tensor(xn[:w, sl], a1[:w],
                                                        x0b[:w, sl], op=ALU.add)
                                nc.sync.dma_start(bounce[l + 1][r0:r0 + w, :],
                                                  xn[:w, sl])
                            else:
                                x4 = work.tile([P, P], f32, name="x4", tag="a3")
                                nc.vector.tensor_tensor(x4[:w], a1[:w],
                                                        x0b[:w, sl], op=ALU.add)
                                tps = z2_ps.tile([P, P], f32, name="tps",
                                                 tag="z2")
                                nc.tensor.transpose(tps[:, :w], x4[:w],
                                                    identity=idn[:w, :w])
                                x4T = work.tile([P, P], DT, name="x4T",
                                                tag="a1")
                                nc.vector.tensor_copy(x4T[:, :w], tps[:, :w])
                                lg = lg_ps.tile([P, cfg.NCLASS], f32,
                                                name="lg", tag="lg")
                                nc.tensor.matmul(lg[:w, :], lhsT=x4T[:, :w],
                                                 rhs=wo_sb[:], start=True,
                                                 stop=True)
                                mx = work.tile([P, 1], f32, name="mx", tag="mx")
                                nc.vector.tensor_reduce(mx[:w], lg[:w, :],
                                                        axis=AX.X, op=ALU.max)
                                nmx = work.tile([P, 1], f32, name="nmx",
                                                tag="nmx")
                                nc.vector.tensor_scalar(nmx[:w], mx[:w], -1.0,
                                                        None, ALU.mult)
                                ex = work.tile([P, cfg.NCLASS], f32, name="ex",
                                               tag="ex")
                                sume = work.tile([P, 1], f32, name="sume",
                                                 tag="sume")
                                nc.scalar.activation(ex[:w], lg[:w, :], AFT.Exp,
                                                     bias=nmx[:w],
                                                     accum_out=sume[:w])
                                lse = work.tile([P, 1], f32, name="lse",
                                                tag="lse")
                                nc.scalar.activation(lse[:w], sume[:w], AFT.Ln)
                                nc.vector.tensor_tensor(lse[:w], lse[:w],
                                                        nmx[:w], op=ALU.subtract)
                                res = work.tile([P, cfg.NCLASS], f32,
                                                name="res", tag="ex")
                                nc.vector.tensor_scalar(res[:w], lg[:w, :],
                                                        lse[:w], None,
                                                        ALU.subtract)
                                nc.sync.dma_start(out[r0:r0 + w, :], res[:w])
                    if not last:
                        if no_coll:
                            nc.sync.dma_start(x_full[l + 1][:R, :],
                                              bounce[l + 1][:])
                        else:
                            nc.gpsimd.collective_compute(
                                "AllGather", ALU.bypass,
                                ins=[bounce[l + 1].opt()],
                                outs=[x_full[l + 1].opt()],
                                replica_groups=rg)
    nc.compile()
    return nc


def kernel(**inputs) -> np.ndarray:
    cfg = Cfg()
    features = np.asarray(inputs["features"], np.float32)
    edge_row = np.asarray(inputs["edge_row"], np.int64)
    edge_col = np.asarray(inputs["edge_col"], np.int64)
    W_in = np.asarray(inputs["W_in"], np.float32)
    Ws = np.asarray(inputs["Ws"], np.float32)
    c = np.asarray(inputs["c"], np.float32)
    W_out = np.asarray(inputs["W_out"], np.float32)

    in_maps, sch, perm = preprocess(cfg, features, edge_row, edge_col,
                                    W_in, Ws, c, W_out)
    nc = build_program(cfg, sch)

    import os
    from concourse import bass_utils
    res = bass_utils.run_bass_kernel_spmd(
        nc, in_maps, core_ids=list(range(cfg.n_cores)),
        trace=bool(os.environ.get("GNN_TRACE")))
    kernel.last_result = res
    out = np.empty((cfg.N, cfg.NCLASS), np.float32)
    for d in range(cfg.n_cores):
        out[perm[d * cfg.R:(d + 1) * cfg.R]] = res.results[d]["out"]
    return out

